# revision 17
# baseline (speedup 1.0000x reference)
"""Trainium2 Bass kernel for ContinuousConv1DSim (gnn_message_passing).

Delta-band bf16 formulation (validated in numpy emulation, rel err ~3e-3):
  Overlapping l-tiles of 128 events at stride 121 (window Ks=9 fully inside
  a tile for lanes p>=7), so no halo accumulation chain.
  G = F * npm_j (pre-masked on host, bf16).
  Per batch, band operands [j, (n,p)] built from gpsimd partition-broadcast
  rows:
    BandD = (tloc_l - tloc_j) * band  (delta-weighted band; tloc centered
                                       per tile so bf16 stays accurate)
    BandU = udt_l * band              (udt = masked dt to next event)
  MM1 (PE, bf16): ptile[c2, p]: SD = G^T @ BandD -> partitions 0:64;
    one merged matmul streams [band | BandU_n] -> SG, SU at partitions
    64:128 (free 0:128 / 128:256).
  MM2 (PE, bf16): obs[p, 512] = [SD|SG]^T @ prbA + [SU]^T @ prbB
    8 col-blocks of 64: block s = SD@W^T + SG@bias + udt*u_s*(SG@W^T)
    == sim row s before npm_l masking (applied as scale on PSUM->SBUF copy).
  Real row for l+1 = nsh * (simbase + udt*A) obtained as a fixed linear
  combination of blocks s=0 and s=7 (coefficients folded into host scalars);
  valid because right-padding makes npm monotone.
  PE loop is software-pipelined: MM1 of tile n+1 issues before MM2 of n,
  and the next batch's DMA/broadcast/band-builds are interleaved into the
  current batch's tile loop.

Pure data parallel: batch 32 -> 8 cores x 4. All params replicated.
"""

import numpy as np

B, L, C, O, S = 32, 2048, 64, 64, 8
NCORES = 8
BPC = B // NCORES            # 4 batches per core
STRIDE = 121                 # l-tile stride (128 - 7 overlap)
NT2 = 17                     # tiles per batch: 121*16 + 128 > 2048
EXT = NT2 * 128              # 2176
ROWS = (L - 1) * (S + 1) + 1  # 18424


def _consts(W, bias, u):
    import ml_dtypes
    n = np.arange(128)
    band = ((n[:, None] >= n[None, :] - 7) & (n[:, None] <= n[None, :]))
    band = band.astype(np.float32)
    WT = W.T.astype(np.float32)
    prbA = np.zeros((128, 512), np.float32)
    prbB = np.zeros((128, 512), np.float32)
    for k in range(8):
        sl = slice(k * 64, (k + 1) * 64)
        prbA[0:64, sl] = WT           # SD rows -> W^T
        prbA[64:128, sl] = bias       # SG rows -> bias
        prbB[64:128, sl] = u[k] * WT  # SU rows
    bf = ml_dtypes.bfloat16
    return band.astype(bf), prbA.astype(bf), prbB.astype(bf)


def _host_prep(times, feats, npm, u):
    """Per-full-batch host tensors (numpy, cheap)."""
    import ml_dtypes
    bf = ml_dtypes.bfloat16
    IDX = (STRIDE * np.arange(NT2))[None, :] + np.arange(128)[:, None]  # [p, n]
    IDXf = IDX.T.reshape(-1)        # [(n, p)] flattened

    G = feats * npm[:, :, None]     # mask padded events
    G_ext = np.concatenate([G, np.zeros((B, 256, C), np.float32)], 1)
    t_ext = np.concatenate([times, np.repeat(times[:, -1:], 256, 1)], 1)
    npm_ext = np.concatenate([npm, np.zeros((B, 256), np.float32)], 1)
    udt = np.zeros((B, L + 256), np.float32)
    udt[:, :L - 1] = (times[:, 1:] - times[:, :-1]) * npm[:, :-1] * npm[:, 1:]

    cen = times[:, STRIDE * np.arange(NT2)]          # [B, n]
    trow = (t_ext[:, IDXf].reshape(B, NT2, 128)
            - cen[:, :, None]).reshape(B, EXT).astype(np.float32)
    udtrow = udt[:, IDXf].astype(bf)                 # [B, EXT]
    # host-built delta band: bandD[b, j, (n, p)] = (tloc_p - tloc_j) * band
    n128 = np.arange(128)
    bandm = ((n128[:, None] >= n128[None, :] - 7)
             & (n128[:, None] <= n128[None, :])).astype(np.float32)
    tl = trow.reshape(B, NT2, 128)
    bdh = (tl[:, :, None, :] - tl[:, :, :, None]) * bandm[None, None]
    bdh = bdh.transpose(0, 2, 1, 3).reshape(B, 128, EXT).astype(bf)
    # real row from blocks s=0, s=7: simbase + udt*A =
    #   (1-lam)*b0 + lam*b7 with lam = (1-u0)/(u7-u0)
    lam = float((1.0 - u[0]) / (u[7] - u[0]))
    nsh = npm_ext[:, IDX + 1]
    # column scalars [p, n]: tloc, npm, nsh*(1-lam), nsh*lam
    scol = np.empty((B, 128, 4 * NT2), np.float32)
    scol[:, :, :NT2] = trow.reshape(B, NT2, 128).transpose(0, 2, 1)
    scol[:, :, NT2:2 * NT2] = npm_ext[:, IDX]
    scol[:, :, 2 * NT2:3 * NT2] = nsh * (1.0 - lam)
    scol[:, :, 3 * NT2:] = nsh * lam
    hostF = G_ext[:, IDX, :].reshape(B, 128, NT2 * C).astype(bf)  # [B,p,(n c)]
    return hostF, bdh, udtrow, scol


def _build_nc():
    import concourse.bass as bass
    import concourse.bacc as bacc
    import concourse.mybir as mybir
    import concourse.tile as tile

    f32 = mybir.dt.float32
    bf16 = mybir.dt.bfloat16
    Copy = mybir.ActivationFunctionType.Copy
    mult = mybir.AluOpType.mult
    add = mybir.AluOpType.add
    sub = mybir.AluOpType.subtract

    nc = bacc.Bacc("TRN2", target_bir_lowering=False, debug=False,
                   num_devices=NCORES)

    FD = nc.dram_tensor("f", [BPC, 128, NT2 * C], bf16, kind="ExternalInput").ap()
    BDH = nc.dram_tensor("bdh", [BPC, 128, EXT], bf16, kind="ExternalInput").ap()
    URD = nc.dram_tensor("udtrow", [BPC, EXT], bf16, kind="ExternalInput").ap()
    SCD = nc.dram_tensor("scol", [BPC, 128, 4 * NT2], f32, kind="ExternalInput").ap()
    BDD = nc.dram_tensor("band", [128, 128], bf16, kind="ExternalInput").ap()
    PAD = nc.dram_tensor("prbA", [128, 512], bf16, kind="ExternalInput").ap()
    PBD = nc.dram_tensor("prbB", [128, 512], bf16, kind="ExternalInput").ap()
    OUTD = nc.dram_tensor("out", [BPC, ROWS, O], f32, kind="ExternalOutput").ap()

    with tile.TileContext(nc) as tc:
        with (
            tc.tile_pool(name="const", bufs=1) as cpool,
            tc.tile_pool(name="rows", bufs=2) as rpool,
            tc.tile_pool(name="rep", bufs=2) as bpool,
            tc.tile_pool(name="bands", bufs=2) as dpool,
            tc.tile_pool(name="bigbu", bufs=2) as gpool,
            tc.tile_pool(name="feat", bufs=2) as fpool,
            tc.tile_pool(name="sbw", bufs=8) as spool,
            tc.tile_pool(name="osb", bufs=6) as opool,
            tc.tile_pool(name="rsb", bufs=6) as lpool,
            tc.tile_pool(name="rt", bufs=6) as tpool,
            tc.tile_pool(name="pt", bufs=5, space=bass.MemorySpace.PSUM) as ppool,
            tc.tile_pool(name="po", bufs=3, space=bass.MemorySpace.PSUM) as qpool,
        ):
            band_t = cpool.tile([128, 128], bf16, tag="band")
            prbA_t = cpool.tile([128, 512], bf16, tag="prbA")
            prbB_t = cpool.tile([128, 512], bf16, tag="prbB")
            zrow = cpool.tile([1, 64], f32, tag="zrow")
            nc.sync.dma_start(band_t[:], BDD)
            nc.sync.dma_start(prbA_t[:], PAD)
            nc.sync.dma_start(prbB_t[:], PBD)
            nc.gpsimd.memset(zrow[:], 0.0)
            bandv = band_t[:].unsqueeze(1).broadcast_to([128, NT2, 128])

            state = {}

            def prep(b, step):
                """Emit prep piece `step` for batch b; returns nothing."""
                st = state.setdefault(b, {})
                if step == 0:
                    st['urow'] = rpool.tile([1, EXT], bf16, tag="urow", name="urow")
                    st['scol'] = rpool.tile([128, 4 * NT2], f32, tag="scol", name="scol")
                    st['fsb'] = fpool.tile([128, NT2 * C], bf16, tag="f", name="fsb")
                    st['bdd'] = dpool.tile([128, EXT], bf16, tag="bd", name="bdd")
                    st['bigbu'] = gpool.tile([128, NT2 * 256], bf16, tag="bigbu", name="bigbu")
                    nc.gpsimd.dma_start(st['urow'][:], URD[b].unsqueeze(0))
                    nc.gpsimd.dma_start(st['scol'][:], SCD[b])
                    nc.gpsimd.dma_start(st['fsb'][:], FD[b])
                    nc.gpsimd.dma_start(st['bdd'][:], BDH[b])
                    nc.gpsimd.dma_start(OUTD[b, 0:1, :], zrow[:])
                    bb = st['bigbu'][:].rearrange("p (n l) -> p n l", l=256)
                    nc.scalar.copy(bb[:, :, 0:128], bandv)
                elif step == 1:
                    st['urep'] = bpool.tile([128, EXT], bf16, tag="urep", name="urep")
                    nc.gpsimd.partition_broadcast(st['urep'][:], st['urow'][:])
                elif step == 2:
                    bb = st['bigbu'][:].rearrange("p (n l) -> p n l", l=256)
                    nc.vector.scalar_tensor_tensor(
                        bb[:, :, 128:256],
                        st['urep'][:].rearrange("p (n l) -> p n l", l=128),
                        1.0, bandv, op0=mult, op1=mult)

            def mm1(b, n):
                st = state[b]
                G_n = st['fsb'][:, n * C:(n + 1) * C]
                ptile = ppool.tile([128, 256], f32, tag="pt")
                # SD -> partitions 0:64 (free 0:128)
                nc.tensor.matmul(ptile[0:64, 0:128], G_n,
                                 st['bdd'][:, n * 128:(n + 1) * 128],
                                 start=True, stop=True)
                # [SG | SU] -> partitions 64:128 (free 0:256), one stream
                nc.tensor.matmul(ptile[64:128, 0:256], G_n,
                                 st['bigbu'][:, n * 256:(n + 1) * 256],
                                 start=True, stop=True)
                sbw = spool.tile([128, 256], bf16, tag="sbw")
                nc.scalar.copy(sbw[:], ptile[:])
                return sbw

            PREP_AT = {1: 0, 5: 1, 8: 2}

            for b in range(BPC):
                if b == 0:
                    for s in range(3):
                        prep(0, s)
                st = state[b]
                scol_t = st['scol']
                sbws = [mm1(b, 0), mm1(b, 1), mm1(b, 2)]
                for n in range(NT2):
                    if b + 1 < BPC and n in PREP_AT:
                        prep(b + 1, PREP_AT[n])
                    if n + 3 < NT2:
                        sbws.append(mm1(b, n + 3))
                    sbw = sbws.pop(0)
                    obs = qpool.tile([128, 512], f32, tag="po")
                    nc.tensor.matmul(obs[:], sbw[:, 0:128], prbA_t[:],
                                     start=True, stop=False)
                    nc.tensor.matmul(obs[:], sbw[64:128, 128:256],
                                     prbB_t[64:128, :],
                                     start=False, stop=True)
                    # npm_l masking via per-partition scale on the copies
                    osb = opool.tile([128, 512], f32, tag="osb")
                    nc.scalar.activation(osb[:, 0:224], obs[:, 0:224], Copy,
                                         scale=scol_t[:, NT2 + n:NT2 + n + 1])
                    nc.vector.tensor_scalar_mul(
                        osb[:, 224:512], obs[:, 224:512],
                        scol_t[:, NT2 + n:NT2 + n + 1])
                    # real row l+1 = nshl*b0 + nshr*b7
                    rt = tpool.tile([128, 64], f32, tag="rt")
                    nc.vector.tensor_scalar_mul(
                        rt[:], obs[:, 0:64],
                        scol_t[:, 2 * NT2 + n:2 * NT2 + n + 1])
                    rsb = lpool.tile([128, 64], f32, tag="rsb")
                    nc.vector.scalar_tensor_tensor(
                        rsb[:], obs[:, 448:512],
                        scol_t[:, 3 * NT2 + n:3 * NT2 + n + 1], rt[:],
                        op0=mult, op1=add)
                    # DMA out
                    p_lo = 0 if n == 0 else 7
                    p_hi = min(127, 2046 - STRIDE * n)
                    npn = p_hi - p_lo + 1
                    sim_dst = bass.AP(
                        OUTD.tensor,
                        (b * ROWS + 9 * (STRIDE * n + p_lo) + 1) * 64,
                        [[9 * 64, npn], [1, 512]])
                    eng = nc.sync if n % 2 == 0 else nc.gpsimd
                    eng.dma_start(sim_dst, osb[p_lo:p_hi + 1, :])
                    real_dst = bass.AP(
                        OUTD.tensor,
                        (b * ROWS + 9 * (STRIDE * n + p_lo + 1)) * 64,
                        [[9 * 64, npn], [1, 64]])
                    nc.gpsimd.dma_start(real_dst, rsb[p_lo:p_hi + 1, :])
                del state[b]
    nc.compile()
    return nc


_NC_CACHE = None


def _in_maps(inputs):
    times = np.ascontiguousarray(inputs["times"], np.float32)
    feats = np.ascontiguousarray(inputs["features"], np.float32)
    npm = inputs["non_pad_mask"].astype(np.float32)
    u = np.asarray(inputs["uniform_sample"], np.float32)
    W = np.ascontiguousarray(inputs["W"], np.float32)
    bias = np.ascontiguousarray(inputs["bias_param"], np.float32)

    band, prbA, prbB = _consts(W, bias, u)
    hostF, bdh, udtrow, scol = _host_prep(times, feats, npm, u)

    in_maps = []
    for c in range(NCORES):
        sl = slice(c * BPC, (c + 1) * BPC)
        in_maps.append({
            "f": np.ascontiguousarray(hostF[sl]),
            "bdh": np.ascontiguousarray(bdh[sl]),
            "udtrow": np.ascontiguousarray(udtrow[sl]),
            "scol": np.ascontiguousarray(scol[sl]),
            "band": band, "prbA": prbA, "prbB": prbB,
        })
    return in_maps


def kernel(**inputs):
    global _NC_CACHE
    from concourse.bass_utils import run_bass_kernel_spmd

    if _NC_CACHE is None:
        _NC_CACHE = _build_nc()
    nc = _NC_CACHE
    in_maps = _in_maps(inputs)
    res = run_bass_kernel_spmd(nc, in_maps, core_ids=list(range(NCORES)))
    out = np.concatenate([r["out"] for r in res.results], 0)
    return out.astype(np.float32)


# revision 18
# speedup vs baseline: 1.0596x; 1.0596x over previous
"""Trainium2 Bass kernel for ContinuousConv1DSim (gnn_message_passing).

Delta-band bf16 formulation (validated in numpy emulation, rel err ~3e-3):
  Overlapping l-tiles of 128 events at stride 121 (window Ks=9 fully inside
  a tile for lanes p>=7), so no halo accumulation chain.
  G = F * npm_j (pre-masked on host, bf16).
  Per batch, band operands [j, (n,p)] built from gpsimd partition-broadcast
  rows:
    BandD = (tloc_l - tloc_j) * band  (delta-weighted band; tloc centered
                                       per tile so bf16 stays accurate)
    BandU = udt_l * band              (udt = masked dt to next event)
  MM1 (PE, bf16): ptile[c2, p]: SD = G^T @ BandD -> partitions 0:64;
    one merged matmul streams [band | BandU_n] -> SG, SU at partitions
    64:128 (free 0:128 / 128:256).
  MM2 (PE, bf16): obs[p, 512] = [SD|SG]^T @ prbA + [SU]^T @ prbB
    8 col-blocks of 64: block s = SD@W^T + SG@bias + udt*u_s*(SG@W^T)
    == sim row s before npm_l masking (applied as scale on PSUM->SBUF copy).
  Real row for l+1 = nsh * (simbase + udt*A) obtained as a fixed linear
  combination of blocks s=0 and s=7 (coefficients folded into host scalars);
  valid because right-padding makes npm monotone.
  PE loop is software-pipelined: MM1 of tile n+1 issues before MM2 of n,
  and the next batch's DMA/broadcast/band-builds are interleaved into the
  current batch's tile loop.

Pure data parallel: batch 32 -> 8 cores x 4. All params replicated.
"""

import numpy as np

B, L, C, O, S = 32, 2048, 64, 64, 8
NCORES = 8
BPC = B // NCORES            # 4 batches per core
STRIDE = 121                 # l-tile stride (128 - 7 overlap)
NT2 = 17                     # tiles per batch: 121*16 + 128 > 2048
EXT = NT2 * 128              # 2176
ROWS = (L - 1) * (S + 1) + 1  # 18424


def _consts(W, bias, u):
    import ml_dtypes
    n = np.arange(128)
    band = ((n[:, None] >= n[None, :] - 7) & (n[:, None] <= n[None, :]))
    band = band.astype(np.float32)
    WT = W.T.astype(np.float32)
    prbA = np.zeros((128, 512), np.float32)
    prbB = np.zeros((128, 512), np.float32)
    for k in range(8):
        sl = slice(k * 64, (k + 1) * 64)
        prbA[0:64, sl] = WT           # SD rows -> W^T
        prbA[64:128, sl] = bias       # SG rows -> bias
        prbB[64:128, sl] = u[k] * WT  # SU rows
    bf = ml_dtypes.bfloat16
    return band.astype(bf), prbA.astype(bf), prbB.astype(bf)


def _host_prep(times, feats, npm, u):
    """Per-full-batch host tensors (numpy, cheap)."""
    import ml_dtypes
    bf = ml_dtypes.bfloat16
    IDX = (STRIDE * np.arange(NT2))[None, :] + np.arange(128)[:, None]  # [p, n]
    IDXf = IDX.T.reshape(-1)        # [(n, p)] flattened

    G = feats * npm[:, :, None]     # mask padded events
    G_ext = np.concatenate([G, np.zeros((B, 256, C), np.float32)], 1)
    t_ext = np.concatenate([times, np.repeat(times[:, -1:], 256, 1)], 1)
    npm_ext = np.concatenate([npm, np.zeros((B, 256), np.float32)], 1)
    udt = np.zeros((B, L + 256), np.float32)
    udt[:, :L - 1] = (times[:, 1:] - times[:, :-1]) * npm[:, :-1] * npm[:, 1:]

    cen = times[:, STRIDE * np.arange(NT2)]          # [B, n]
    trow = (t_ext[:, IDXf].reshape(B, NT2, 128)
            - cen[:, :, None]).reshape(B, EXT).astype(np.float32)
    udtrow = udt[:, IDXf].astype(bf)                 # [B, EXT]
    # host-built delta band: bandD[b, j, (n, p)] = (tloc_p - tloc_j) * band
    n128 = np.arange(128)
    bandm = ((n128[:, None] >= n128[None, :] - 7)
             & (n128[:, None] <= n128[None, :])).astype(np.float32)
    tl = trow.reshape(B, NT2, 128)
    bdh = (tl[:, :, None, :] - tl[:, :, :, None]) * bandm[None, None]
    bdh = bdh.transpose(0, 2, 1, 3).reshape(B, 128, EXT).astype(bf)
    # real row from blocks s=0, s=7: simbase + udt*A =
    #   (1-lam)*b0 + lam*b7 with lam = (1-u0)/(u7-u0)
    lam = float((1.0 - u[0]) / (u[7] - u[0]))
    nsh = npm_ext[:, IDX + 1]
    # column scalars [p, n]: tloc, npm, nsh*(1-lam), nsh*lam
    scol = np.empty((B, 128, 4 * NT2), np.float32)
    scol[:, :, :NT2] = trow.reshape(B, NT2, 128).transpose(0, 2, 1)
    scol[:, :, NT2:2 * NT2] = npm_ext[:, IDX]
    scol[:, :, 2 * NT2:3 * NT2] = nsh * (1.0 - lam)
    scol[:, :, 3 * NT2:] = nsh * lam
    hostF = G_ext[:, IDX, :].reshape(B, 128, NT2 * C).astype(bf)  # [B,p,(n c)]
    return hostF, bdh, udtrow, scol


def _build_nc():
    import concourse.bass as bass
    import concourse.bacc as bacc
    import concourse.mybir as mybir
    import concourse.tile as tile

    f32 = mybir.dt.float32
    bf16 = mybir.dt.bfloat16
    Copy = mybir.ActivationFunctionType.Copy
    mult = mybir.AluOpType.mult
    add = mybir.AluOpType.add
    sub = mybir.AluOpType.subtract

    nc = bacc.Bacc("TRN2", target_bir_lowering=False, debug=False,
                   num_devices=NCORES)

    FD = nc.dram_tensor("f", [BPC, 128, NT2 * C], bf16, kind="ExternalInput").ap()
    BDH = nc.dram_tensor("bdh", [BPC, 128, EXT], bf16, kind="ExternalInput").ap()
    URD = nc.dram_tensor("udtrow", [BPC, EXT], bf16, kind="ExternalInput").ap()
    SCD = nc.dram_tensor("scol", [BPC, 128, 4 * NT2], f32, kind="ExternalInput").ap()
    BDD = nc.dram_tensor("band", [128, 128], bf16, kind="ExternalInput").ap()
    PAD = nc.dram_tensor("prbA", [128, 512], bf16, kind="ExternalInput").ap()
    PBD = nc.dram_tensor("prbB", [128, 512], bf16, kind="ExternalInput").ap()
    OUTD = nc.dram_tensor("out", [BPC, ROWS, O], f32, kind="ExternalOutput").ap()

    with tile.TileContext(nc) as tc:
        with (
            tc.tile_pool(name="const", bufs=1) as cpool,
            tc.tile_pool(name="rows", bufs=2) as rpool,
            tc.tile_pool(name="rep", bufs=2) as bpool,
            tc.tile_pool(name="bands", bufs=2) as dpool,
            tc.tile_pool(name="bigbu", bufs=2) as gpool,
            tc.tile_pool(name="feat", bufs=2) as fpool,
            tc.tile_pool(name="sbw", bufs=8) as spool,
            tc.tile_pool(name="osb", bufs=6) as opool,
            tc.tile_pool(name="rsb", bufs=6) as lpool,
            tc.tile_pool(name="rt", bufs=6) as tpool,
            tc.tile_pool(name="pt", bufs=5, space=bass.MemorySpace.PSUM) as ppool,
            tc.tile_pool(name="po", bufs=3, space=bass.MemorySpace.PSUM) as qpool,
        ):
            band_t = cpool.tile([128, 128], bf16, tag="band")
            prbA_t = cpool.tile([128, 512], bf16, tag="prbA")
            prbB_t = cpool.tile([128, 512], bf16, tag="prbB")
            zrow = cpool.tile([1, 64], f32, tag="zrow")
            nc.sync.dma_start(band_t[:], BDD)
            nc.sync.dma_start(prbA_t[:], PAD)
            nc.sync.dma_start(prbB_t[:], PBD)
            nc.gpsimd.memset(zrow[:], 0.0)
            bandv = band_t[:].unsqueeze(1).broadcast_to([128, NT2, 128])

            state = {}

            def prep(b, step):
                """Emit prep piece `step` for batch b; returns nothing."""
                st = state.setdefault(b, {})
                if step == 0:
                    st['urow'] = rpool.tile([1, EXT], bf16, tag="urow", name="urow")
                    st['scol'] = rpool.tile([128, 4 * NT2], f32, tag="scol", name="scol")
                    st['fsb'] = fpool.tile([128, NT2 * C], bf16, tag="f", name="fsb")
                    st['bdd'] = dpool.tile([128, EXT], bf16, tag="bd", name="bdd")
                    st['bigbu'] = gpool.tile([128, NT2 * 256], bf16, tag="bigbu", name="bigbu")
                    nc.gpsimd.dma_start(st['urow'][:], URD[b].unsqueeze(0))
                    nc.gpsimd.dma_start(st['scol'][:], SCD[b])
                    nc.gpsimd.dma_start(st['fsb'][:], FD[b])
                    nc.gpsimd.dma_start(st['bdd'][:], BDH[b])
                    nc.gpsimd.dma_start(OUTD[b, 0:1, :], zrow[:])
                    bb = st['bigbu'][:].rearrange("p (n l) -> p n l", l=256)
                    nc.scalar.copy(bb[:, :, 0:128], bandv)
                elif step == 1:
                    st['urep'] = bpool.tile([128, EXT], bf16, tag="urep", name="urep")
                    nc.gpsimd.partition_broadcast(st['urep'][:], st['urow'][:])
                elif step == 2:
                    bb = st['bigbu'][:].rearrange("p (n l) -> p n l", l=256)
                    nc.vector.scalar_tensor_tensor(
                        bb[:, :, 128:256],
                        st['urep'][:].rearrange("p (n l) -> p n l", l=128),
                        1.0, bandv, op0=mult, op1=mult)

            def mm1(b, n):
                st = state[b]
                G_n = st['fsb'][:, n * C:(n + 1) * C]
                ptile = ppool.tile([128, 256], f32, tag="pt")
                # SD -> partitions 0:64 (free 0:128)
                nc.tensor.matmul(ptile[0:64, 0:128], G_n,
                                 st['bdd'][:, n * 128:(n + 1) * 128],
                                 start=True, stop=True)
                # [SG | SU] -> partitions 64:128 (free 0:256), one stream
                nc.tensor.matmul(ptile[64:128, 0:256], G_n,
                                 st['bigbu'][:, n * 256:(n + 1) * 256],
                                 start=True, stop=True)
                sbw = spool.tile([128, 256], bf16, tag="sbw")
                nc.scalar.copy(sbw[:], ptile[:])
                return sbw

            PREP_AT = {1: 0, 5: 1, 8: 2}

            for b in range(BPC):
                if b == 0:
                    for s in range(3):
                        prep(0, s)
                st = state[b]
                scol_t = st['scol']
                sbws = [mm1(b, 0), mm1(b, 1), mm1(b, 2)]
                for n in range(NT2):
                    if b + 1 < BPC and n in PREP_AT:
                        prep(b + 1, PREP_AT[n])
                    if n + 3 < NT2:
                        sbws.append(mm1(b, n + 3))
                    sbw = sbws.pop(0)
                    obs = qpool.tile([128, 512], f32, tag="po")
                    nc.tensor.matmul(obs[:], sbw[:, 0:128], prbA_t[:],
                                     start=True, stop=False)
                    nc.tensor.matmul(obs[:], sbw[64:128, 128:256],
                                     prbB_t[64:128, :],
                                     start=False, stop=True)
                    # npm_l masking via per-partition scale on the copies
                    osb = opool.tile([128, 512], f32, tag="osb")
                    nc.scalar.activation(osb[:, 0:224], obs[:, 0:224], Copy,
                                         scale=scol_t[:, NT2 + n:NT2 + n + 1])
                    nc.vector.tensor_scalar_mul(
                        osb[:, 224:512], obs[:, 224:512],
                        scol_t[:, NT2 + n:NT2 + n + 1])
                    # real row l+1 = nshl*b0 + nshr*b7
                    rt = tpool.tile([128, 64], f32, tag="rt")
                    nc.vector.tensor_scalar_mul(
                        rt[:], obs[:, 0:64],
                        scol_t[:, 2 * NT2 + n:2 * NT2 + n + 1])
                    rsb = lpool.tile([128, 64], f32, tag="rsb")
                    nc.vector.scalar_tensor_tensor(
                        rsb[:], obs[:, 448:512],
                        scol_t[:, 3 * NT2 + n:3 * NT2 + n + 1], rt[:],
                        op0=mult, op1=add)
                    # DMA out
                    p_lo = 0 if n == 0 else 7
                    p_hi = min(127, 2046 - STRIDE * n)
                    npn = p_hi - p_lo + 1
                    sim_dst = bass.AP(
                        OUTD.tensor,
                        (b * ROWS + 9 * (STRIDE * n + p_lo) + 1) * 64,
                        [[9 * 64, npn], [1, 512]])
                    nc.sync.dma_start(sim_dst, osb[p_lo:p_hi + 1, :])
                    real_dst = bass.AP(
                        OUTD.tensor,
                        (b * ROWS + 9 * (STRIDE * n + p_lo + 1)) * 64,
                        [[9 * 64, npn], [1, 64]])
                    nc.gpsimd.dma_start(real_dst, rsb[p_lo:p_hi + 1, :])
                del state[b]
    nc.compile()
    return nc


_NC_CACHE = None


def _in_maps(inputs):
    times = np.ascontiguousarray(inputs["times"], np.float32)
    feats = np.ascontiguousarray(inputs["features"], np.float32)
    npm = inputs["non_pad_mask"].astype(np.float32)
    u = np.asarray(inputs["uniform_sample"], np.float32)
    W = np.ascontiguousarray(inputs["W"], np.float32)
    bias = np.ascontiguousarray(inputs["bias_param"], np.float32)

    band, prbA, prbB = _consts(W, bias, u)
    hostF, bdh, udtrow, scol = _host_prep(times, feats, npm, u)

    in_maps = []
    for c in range(NCORES):
        sl = slice(c * BPC, (c + 1) * BPC)
        in_maps.append({
            "f": np.ascontiguousarray(hostF[sl]),
            "bdh": np.ascontiguousarray(bdh[sl]),
            "udtrow": np.ascontiguousarray(udtrow[sl]),
            "scol": np.ascontiguousarray(scol[sl]),
            "band": band, "prbA": prbA, "prbB": prbB,
        })
    return in_maps


def kernel(**inputs):
    global _NC_CACHE
    from concourse.bass_utils import run_bass_kernel_spmd

    if _NC_CACHE is None:
        _NC_CACHE = _build_nc()
    nc = _NC_CACHE
    in_maps = _in_maps(inputs)
    res = run_bass_kernel_spmd(nc, in_maps, core_ids=list(range(NCORES)))
    out = np.concatenate([r["out"] for r in res.results], 0)
    return out.astype(np.float32)
